# revision 22
# baseline (speedup 1.0000x reference)
"""Trainium2 Bass kernel for the DPAAUser3D segment-reduce problem.

Computes, for x[B=2,C=8,D=H=W=128] and attentions[B,C,512,1]:
  onehot = one_hot(argmax_c x)                      (per-voxel channel argmax)
  adj    = avgpool_8x8x8(onehot)                    ([B,C,16,16,16], = counts/512)
  corr[b,c,d,h,w] = att[b,c,(d//16*8+h//16)*8+w//16] * adj[b,c,d%16,h%16,w%16]
  out1   = x * (1+corr)^2
  out2   = corr

Sharding: data-parallel over D (16 slices per core, 8 cores). Pooling is
D-local; one 8KB AllGather per batch element distributes the pooled counts.

Single pass over x: the host pre-transposes each core's slice to
[B, H, DL, C, W] so every load/store is a >=1MB DMA with 16KB-contiguous
rows (H on partitions, which the pooling matmul needs anyway). The argmax
compare runs on the f32 chunk right after load; a bf16 copy of x stays
resident in SBUF for the output multiply, and all phase-2 elementwise work
(corr, (1+corr)^2, x*(...)) runs in bf16 so the DVE's 2x 16-bit mode
applies. Outputs are written as bf16 (harness gate is rel_err < 2e-2;
measured end-to-end error of this scheme is ~5e-3). Per-core HBM traffic:
16.8MB read + 16.8MB written vs 67MB for the two-pass f32 version.
"""

import sys

import numpy as np

try:
    import concourse.bass as bass
except ImportError:  # fresh grading dir: concourse lives in the repo checkout
    for p in ("/opt/trn_rl_repo", "/root/.axon_site/_ro/trn_rl_repo"):
        if p not in sys.path:
            sys.path.insert(0, p)
    import concourse.bass as bass

import ml_dtypes
import concourse.bacc as bacc
import concourse.mybir as mybir
import concourse.tile as tile
from concourse import bass_utils

B, C, D, H, W = 2, 8, 128, 128, 128
POOL = 8          # pooling block edge
PATCH = 16        # fold patch edge
G = D // PATCH    # 8 patches per spatial dim
NCORES = 8
DL = D // NCORES  # 16 d-slices per core
PD = DL // POOL   # 2 pooled kd-blocks per core
CH = 4            # d-slices per DMA chunk
NCH = DL // CH    # 4 chunks per batch element

F32 = mybir.dt.float32
BF16 = mybir.dt.bfloat16

_CACHE = {}


def _build_nc():
    nc = bacc.Bacc("TRN2", target_bir_lowering=False, debug=False,
                   num_devices=NCORES)

    # x transposed on host: [b, h, dl, c, w] (h on partitions)
    xt = nc.dram_tensor("xt", [B, H, DL, C, W], F32, kind="ExternalInput").ap()
    # att2x[q=(ph,kh), b, c, pw, kw] = att[b,c, core*64 + ph*8 + pw] / 512
    # (pre-expanded over kw so the corr multiply has packed bf16 operands)
    att2x = nc.dram_tensor("att2x", [128, B, C, G, PATCH], BF16,
                           kind="ExternalInput").ap()
    pmat = nc.dram_tensor("pmat", [H, PATCH], BF16, kind="ExternalInput").ap()
    o1c = nc.dram_tensor("o1c", [B, H, DL, C, W], BF16, kind="ExternalOutput").ap()
    o2c = nc.dram_tensor("o2c", [B, H, DL, C, W], BF16, kind="ExternalOutput").ap()

    with tile.TileContext(nc) as tc:
        with (
            tc.tile_pool(name="big", bufs=1) as big,
            tc.tile_pool(name="p1", bufs=2) as p1,
            tc.tile_pool(name="p2", bufs=2) as p2,
            tc.tile_pool(name="psum", bufs=1, space="PSUM") as pp,
            tc.tile_pool(name="dram", bufs=1, space="DRAM") as dram,
        ):
            Pm = big.tile([128, PATCH], BF16, name="Pm")
            At = big.tile([128, B, C, G, PATCH], BF16, name="At")

            # per-(pd,hf) pooled counts accumulate here; reused across b
            psums = {}
            for pd in range(PD):
                for hf in range(2):
                    psums[(pd, hf)] = pp.tile([16, 512], F32,
                                              name=f"ps{pd}{hf}",
                                              tag=f"ps{pd}{hf}")

            # payload layout [pd][kh][c][kw]; gathered flat = [kd][kh][c][kw]
            # (one 8KB AllGather per b, fired as soon as that b's map is done)
            adj_in = [dram.tile([PD, PATCH, C, PATCH], BF16, name=f"adj_in{b}")
                      for b in range(B)]
            adj_gat = [dram.tile([NCORES, PD, PATCH, C, PATCH], BF16,
                                 name=f"adj_gat{b}", addr_space="Shared")
                       for b in range(B)]
            # AdjRep[q=(ph,kh), dl, c, kw] = counts[b, c, dl, kh, kw]
            AdjRep = [big.tile([128, DL, C, PATCH], BF16, name=f"AdjRep{b}")
                      for b in range(B)]

            # bf16 copy of x, resident between phases (8 chunk tiles, 8MB)
            Xb = {}
            for b in range(B):
                for ch in range(NCH):
                    Xb[(b, ch)] = big.tile([128, CH, C, W], BF16,
                                           name=f"xb{b}_{ch}", tag=f"xb{b}_{ch}")

            # ---- phase 1: argmax one-hot + pooled counts ----
            first = True
            for b in range(B):
                for ch in range(NCH):
                    Xc = p1.tile([128, CH, C, W], F32, name="xc", tag="xc",
                                 bufs=4)
                    nc.sync.dma_start(out=Xc, in_=xt[b][:, ch * CH:(ch + 1) * CH])
                    if first:
                        # small constant loads go after the first x chunk so
                        # they don't delay the DVE pipeline start
                        nc.sync.dma_start(out=Pm, in_=pmat)
                        nc.sync.dma_start(out=At, in_=att2x)
                        first = False
                    t1 = p1.tile([128, CH, 4, W], F32, name="t1", tag="t1",
                                 bufs=1)
                    nc.vector.tensor_max(t1, Xc[:, :, 0:4], Xc[:, :, 4:8])
                    t2 = p1.tile([128, CH, 2, W], F32, name="t2", tag="t2",
                                 bufs=1)
                    nc.vector.tensor_max(t2, t1[:, :, 0:2], t1[:, :, 2:4])
                    M = p1.tile([128, CH, W], F32, name="M", tag="M", bufs=1)
                    nc.vector.tensor_max(M, t2[:, :, 0], t2[:, :, 1])
                    eq = p1.tile([128, CH, C, W], BF16, name="eq", tag="eq")
                    nc.vector.tensor_tensor(
                        eq, Xc, M.unsqueeze(2).broadcast_to([128, CH, C, W]),
                        op=mybir.AluOpType.is_equal)
                    # resident bf16 x for the phase-2 output multiply
                    nc.scalar.copy(out=Xb[(b, ch)], in_=Xc)
                    for i in range(CH):
                        dl = ch * CH + i
                        eqf = eq[:, i].rearrange("p c w -> p (c w)")
                        pd, dd = dl // POOL, dl % POOL
                        for hf in range(2):
                            nc.tensor.matmul(psums[(pd, hf)], lhsT=Pm,
                                             rhs=eqf[:, hf * 512:(hf + 1) * 512],
                                             start=(dd == 0),
                                             stop=(dd == POOL - 1))
                        if dd == POOL - 1:
                            # bf16 reduce: DVE accumulates internally in f32,
                            # the integer count (<=512) rounds once on write
                            # (exactly the verified error model)
                            adjpb = p1.tile([16, C, PATCH], BF16, name="adjpb",
                                            tag="adjpb")
                            with nc.allow_low_precision(
                                    reason="integer counts <=512, one rounding"):
                                for hf in range(2):
                                    src = psums[(pd, hf)].rearrange(
                                        "p (c wb wi) -> p c wb wi",
                                        c=4, wb=16, wi=8)
                                    nc.vector.reduce_sum(
                                        adjpb[:, hf * 4:(hf + 1) * 4, :], src,
                                        axis=mybir.AxisListType.X)
                            # scalar-ring store: head-blocking there only
                            # delays xb copies (not needed until phase 2),
                            # and it keeps the gpsimd queue a pure AG chain
                            nc.scalar.dma_start(out=adj_in[b][pd], in_=adjpb)
                nc.gpsimd.collective_compute(
                    "AllGather", mybir.AluOpType.bypass,
                    replica_groups=[list(range(NCORES))],
                    ins=[adj_in[b].opt()], outs=[adj_gat[b].opt()])

            # ---- phase 2: correction + outputs (bf16, 2x DVE mode) ----
            for b in range(B):
                # replicated read of gathered rows: row kd=dl lives in buffer
                # (b, dl%2) at slot dl//2; partitions (ph,kh), ph replicated.
                # Issued here (not earlier) so these scalar-ring waits never
                # block phase-1 scalar work.
                for dl in range(DL):
                    rep = bass.AP(tensor=adj_gat[b].tensor,
                                  offset=adj_gat[b].offset + dl * PATCH * C * PATCH,
                                  ap=[[0, POOL], [C * PATCH, PATCH],
                                      [1, C * PATCH]])
                    nc.scalar.dma_start(
                        out=AdjRep[b][:, dl].rearrange("p c k -> p (c k)"),
                        in_=rep)
                a_b = At[:, b]  # [128, C, G, PATCH] packed bf16
                for ch in range(NCH):
                    Cc = p2.tile([128, CH, C, W], BF16, name="corr", tag="corr")
                    O1 = p2.tile([128, CH, C, W], BF16, name="o1t", tag="o1t")
                    # all corrs first, then u2s, then o1ts: the in-order DVE
                    # queue never waits on the ACT round-trip this way
                    for i in range(CH):
                        dl = ch * CH + i
                        corr_s = Cc[:, i].rearrange("p c (g k) -> p c g k", g=G)
                        r_b = AdjRep[b][:, dl].unsqueeze(2).broadcast_to(
                            [128, C, G, PATCH])
                        nc.vector.tensor_mul(corr_s, a_b, r_b)
                    u2s = []
                    for i in range(CH):
                        u2 = p2.tile([128, C * W], BF16, name="u2", tag="u2",
                                     bufs=2)
                        u2s.append(u2)
                        nc.scalar.activation(
                            u2, Cc[:, i].rearrange("p c w -> p (c w)"),
                            mybir.ActivationFunctionType.Square,
                            bias=1.0, scale=1.0)
                    for i in range(CH):
                        nc.vector.tensor_mul(
                            O1[:, i].rearrange("p c w -> p (c w)"),
                            Xb[(b, ch)][:, i].rearrange("p c w -> p (c w)"),
                            u2s[i])
                    sl = slice(ch * CH, (ch + 1) * CH)
                    # split across the two HWDGE rings: two concurrent streams
                    nc.scalar.dma_start(out=o2c[b][:, sl], in_=Cc)
                    nc.sync.dma_start(out=o1c[b][:, sl], in_=O1)

    nc.compile()
    return nc


def _fix_ties(x):
    """The device one-hot marks every channel equal to the max; the reference
    one_hot(argmax) marks only the first. Nudge later tied channels down by
    one ulp so a plain equality compare reproduces first-match semantics
    (out1 changes by <=1 ulp at those voxels)."""
    mx = x.max(axis=1, keepdims=True)
    ties = x == mx
    multi = ties.sum(axis=1) > 1
    if not multi.any():
        return x
    x = x.copy()
    for b, d, h, w in np.argwhere(multi):
        cs = np.flatnonzero(ties[b, :, d, h, w])
        for c in cs[1:]:
            x[b, c, d, h, w] = np.nextafter(x[b, c, d, h, w], -np.inf)
    return x


def _host_inputs(x, attentions):
    """Build per-core input maps from full inputs."""
    x = _fix_ties(x)
    att = attentions[..., 0].astype(np.float32) * np.float32(1.0 / 512.0)
    att_p = att.reshape(B, C, G, G, G).astype(ml_dtypes.bfloat16)
    pm = np.zeros((H, PATCH), dtype=ml_dtypes.bfloat16)
    pm[np.arange(H), np.arange(H) // POOL] = 1.0

    in_maps = []
    for core in range(NCORES):
        xs = x[:, :, core * DL:(core + 1) * DL]
        xt = np.ascontiguousarray(xs.transpose(0, 3, 2, 1, 4))  # [b,h,dl,c,w]
        # att2x[(ph,kh), b, c, pw, kw] = att_p[b, c, core, ph, pw]
        a = att_p[:, :, core]                         # [B, C, ph, pw]
        a2 = np.ascontiguousarray(np.broadcast_to(
            a.transpose(2, 0, 1, 3)[:, None, :, :, :, None],
            (G, PATCH, B, C, G, PATCH)).reshape(128, B, C, G, PATCH))
        in_maps.append({"xt": xt, "att2x": a2, "pmat": pm})
    return in_maps


def kernel(x, attentions):
    x = np.asarray(x, dtype=np.float32)
    attentions = np.asarray(attentions, dtype=np.float32)

    if "nc" not in _CACHE:
        _CACHE["nc"] = _build_nc()
    nc = _CACHE["nc"]

    in_maps = _host_inputs(x, attentions)
    res = bass_utils.run_bass_kernel_spmd(nc, in_maps,
                                          core_ids=list(range(NCORES)))

    out1 = np.empty((B, C, D, H, W), np.float32)
    out2 = np.empty((B, C, D, H, W), np.float32)
    for core in range(NCORES):
        sl = slice(core * DL, (core + 1) * DL)
        # [b,h,dl,c,w] -> [b,c,dl,h,w]
        out1[:, :, sl] = res.results[core]["o1c"].transpose(
            0, 3, 2, 1, 4).astype(np.float32)
        out2[:, :, sl] = res.results[core]["o2c"].transpose(
            0, 3, 2, 1, 4).astype(np.float32)
    return out1, out2


# revision 23
# speedup vs baseline: 1.0163x; 1.0163x over previous
"""Trainium2 Bass kernel for the DPAAUser3D segment-reduce problem.

Computes, for x[B=2,C=8,D=H=W=128] and attentions[B,C,512,1]:
  onehot = one_hot(argmax_c x)                      (per-voxel channel argmax)
  adj    = avgpool_8x8x8(onehot)                    ([B,C,16,16,16], = counts/512)
  corr[b,c,d,h,w] = att[b,c,(d//16*8+h//16)*8+w//16] * adj[b,c,d%16,h%16,w%16]
  out1   = x * (1+corr)^2
  out2   = corr

Sharding: data-parallel over D (16 slices per core, 8 cores). Pooling is
D-local; one 8KB AllGather per batch element distributes the pooled counts.

Single pass over x: the host pre-transposes each core's slice to
[B, H, DL, C, W] so every load/store is a >=1MB DMA with 16KB-contiguous
rows (H on partitions, which the pooling matmul needs anyway). The argmax
compare runs on the f32 chunk right after load; a bf16 copy of x stays
resident in SBUF for the output multiply, and all phase-2 elementwise work
(corr, (1+corr)^2, x*(...)) runs in bf16 so the DVE's 2x 16-bit mode
applies. Outputs are written as bf16 (harness gate is rel_err < 2e-2;
measured end-to-end error of this scheme is ~5e-3). Per-core HBM traffic:
16.8MB read + 16.8MB written vs 67MB for the two-pass f32 version.
"""

import sys

import numpy as np

try:
    import concourse.bass as bass
except ImportError:  # fresh grading dir: concourse lives in the repo checkout
    for p in ("/opt/trn_rl_repo", "/root/.axon_site/_ro/trn_rl_repo"):
        if p not in sys.path:
            sys.path.insert(0, p)
    import concourse.bass as bass

import ml_dtypes
import concourse.bacc as bacc
import concourse.mybir as mybir
import concourse.tile as tile
from concourse import bass_utils

B, C, D, H, W = 2, 8, 128, 128, 128
POOL = 8          # pooling block edge
PATCH = 16        # fold patch edge
G = D // PATCH    # 8 patches per spatial dim
NCORES = 8
DL = D // NCORES  # 16 d-slices per core
PD = DL // POOL   # 2 pooled kd-blocks per core
CH = 4            # d-slices per DMA chunk
NCH = DL // CH    # 4 chunks per batch element

F32 = mybir.dt.float32
BF16 = mybir.dt.bfloat16

_CACHE = {}


def _build_nc():
    nc = bacc.Bacc("TRN2", target_bir_lowering=False, debug=False,
                   num_devices=NCORES)

    # x transposed on host: [b, h, dl, c, w] (h on partitions)
    xt = nc.dram_tensor("xt", [B, H, DL, C, W], F32, kind="ExternalInput").ap()
    # att2x[q=(ph,kh), b, c, pw, kw] = att[b,c, core*64 + ph*8 + pw] / 512
    # (pre-expanded over kw so the corr multiply has packed bf16 operands)
    att2x = nc.dram_tensor("att2x", [128, B, C, G, PATCH], BF16,
                           kind="ExternalInput").ap()
    pmat = nc.dram_tensor("pmat", [H, PATCH], BF16, kind="ExternalInput").ap()
    o1c = nc.dram_tensor("o1c", [B, H, DL, C, W], BF16, kind="ExternalOutput").ap()
    o2c = nc.dram_tensor("o2c", [B, H, DL, C, W], BF16, kind="ExternalOutput").ap()

    with tile.TileContext(nc) as tc:
        with (
            tc.tile_pool(name="big", bufs=1) as big,
            tc.tile_pool(name="p1", bufs=2) as p1,
            tc.tile_pool(name="p2", bufs=2) as p2,
            tc.tile_pool(name="psum", bufs=1, space="PSUM") as pp,
            tc.tile_pool(name="dram", bufs=1, space="DRAM") as dram,
        ):
            Pm = big.tile([128, PATCH], BF16, name="Pm")
            At = big.tile([128, B, C, G, PATCH], BF16, name="At")

            # per-(pd,hf) pooled counts accumulate here; reused across b
            psums = {}
            for pd in range(PD):
                for hf in range(2):
                    psums[(pd, hf)] = pp.tile([16, 512], F32,
                                              name=f"ps{pd}{hf}",
                                              tag=f"ps{pd}{hf}")

            # payload layout [pd][kh][c][kw]; gathered flat = [kd][kh][c][kw]
            # (one 8KB AllGather per b, fired as soon as that b's map is done)
            adj_in = [dram.tile([PD, PATCH, C, PATCH], BF16, name=f"adj_in{b}")
                      for b in range(B)]
            adj_gat = [dram.tile([NCORES, PD, PATCH, C, PATCH], BF16,
                                 name=f"adj_gat{b}", addr_space="Shared")
                       for b in range(B)]
            # AdjRep[q=(ph,kh), dl, c, kw] = counts[b, c, dl, kh, kw]
            AdjRep = [big.tile([128, DL, C, PATCH], BF16, name=f"AdjRep{b}")
                      for b in range(B)]

            # bf16 copy of x, resident between phases (8 chunk tiles, 8MB)
            Xb = {}
            for b in range(B):
                for ch in range(NCH):
                    Xb[(b, ch)] = big.tile([128, CH, C, W], BF16,
                                           name=f"xb{b}_{ch}", tag=f"xb{b}_{ch}")

            # ---- phase 1: argmax one-hot + pooled counts ----
            first = True
            for b in range(B):
                for ch in range(NCH):
                    Xc = p1.tile([128, CH, C, W], F32, name="xc", tag="xc",
                                 bufs=3)
                    nc.sync.dma_start(out=Xc, in_=xt[b][:, ch * CH:(ch + 1) * CH])
                    if first:
                        # small constant loads go after the first x chunk so
                        # they don't delay the DVE pipeline start
                        nc.sync.dma_start(out=Pm, in_=pmat)
                        nc.sync.dma_start(out=At, in_=att2x)
                        first = False
                    t1 = p1.tile([128, CH, 4, W], F32, name="t1", tag="t1",
                                 bufs=1)
                    nc.vector.tensor_max(t1, Xc[:, :, 0:4], Xc[:, :, 4:8])
                    t2 = p1.tile([128, CH, 2, W], F32, name="t2", tag="t2",
                                 bufs=1)
                    nc.vector.tensor_max(t2, t1[:, :, 0:2], t1[:, :, 2:4])
                    M = p1.tile([128, CH, W], F32, name="M", tag="M", bufs=1)
                    nc.vector.tensor_max(M, t2[:, :, 0], t2[:, :, 1])
                    eq = p1.tile([128, CH, C, W], BF16, name="eq", tag="eq")
                    nc.vector.tensor_tensor(
                        eq, Xc, M.unsqueeze(2).broadcast_to([128, CH, C, W]),
                        op=mybir.AluOpType.is_equal)
                    # resident bf16 x for the phase-2 output multiply
                    nc.scalar.copy(out=Xb[(b, ch)], in_=Xc)
                    for i in range(CH):
                        dl = ch * CH + i
                        eqf = eq[:, i].rearrange("p c w -> p (c w)")
                        pd, dd = dl // POOL, dl % POOL
                        for hf in range(2):
                            nc.tensor.matmul(psums[(pd, hf)], lhsT=Pm,
                                             rhs=eqf[:, hf * 512:(hf + 1) * 512],
                                             start=(dd == 0),
                                             stop=(dd == POOL - 1))
                        if dd == POOL - 1:
                            # bf16 reduce: DVE accumulates internally in f32,
                            # the integer count (<=512) rounds once on write
                            # (exactly the verified error model)
                            adjpb = p1.tile([16, C, PATCH], BF16, name="adjpb",
                                            tag="adjpb")
                            with nc.allow_low_precision(
                                    reason="integer counts <=512, one rounding"):
                                for hf in range(2):
                                    src = psums[(pd, hf)].rearrange(
                                        "p (c wb wi) -> p c wb wi",
                                        c=4, wb=16, wi=8)
                                    nc.vector.reduce_sum(
                                        adjpb[:, hf * 4:(hf + 1) * 4, :], src,
                                        axis=mybir.AxisListType.X)
                            # scalar-ring store: head-blocking there only
                            # delays xb copies (not needed until phase 2),
                            # and it keeps the gpsimd queue a pure AG chain
                            nc.scalar.dma_start(out=adj_in[b][pd], in_=adjpb)
                nc.gpsimd.collective_compute(
                    "AllGather", mybir.AluOpType.bypass,
                    replica_groups=[list(range(NCORES))],
                    ins=[adj_in[b].opt()], outs=[adj_gat[b].opt()])

            # ---- phase 2: correction + outputs (bf16, 2x DVE mode) ----
            for b in range(B):
                # replicated read of gathered rows: row kd=dl lives in buffer
                # (b, dl%2) at slot dl//2; partitions (ph,kh), ph replicated.
                # Issued here (not earlier) so these scalar-ring waits never
                # block phase-1 scalar work.
                for dl in range(DL):
                    rep = bass.AP(tensor=adj_gat[b].tensor,
                                  offset=adj_gat[b].offset + dl * PATCH * C * PATCH,
                                  ap=[[0, POOL], [C * PATCH, PATCH],
                                      [1, C * PATCH]])
                    nc.scalar.dma_start(
                        out=AdjRep[b][:, dl].rearrange("p c k -> p (c k)"),
                        in_=rep)
                a_b = At[:, b]  # [128, C, G, PATCH] packed bf16
                for ch in range(NCH):
                    Cc = p2.tile([128, CH, C, W], BF16, name="corr", tag="corr")
                    O1 = p2.tile([128, CH, C, W], BF16, name="o1t", tag="o1t")
                    # all corrs first, then u2s, then o1ts: the in-order DVE
                    # queue never waits on the ACT round-trip this way
                    for i in range(CH):
                        dl = ch * CH + i
                        corr_s = Cc[:, i].rearrange("p c (g k) -> p c g k", g=G)
                        r_b = AdjRep[b][:, dl].unsqueeze(2).broadcast_to(
                            [128, C, G, PATCH])
                        nc.vector.tensor_mul(corr_s, a_b, r_b)
                    u2s = []
                    for i in range(CH):
                        u2 = p2.tile([128, C * W], BF16, name="u2", tag="u2",
                                     bufs=5)
                        u2s.append(u2)
                        nc.scalar.activation(
                            u2, Cc[:, i].rearrange("p c w -> p (c w)"),
                            mybir.ActivationFunctionType.Square,
                            bias=1.0, scale=1.0)
                    for i in range(CH):
                        nc.vector.tensor_mul(
                            O1[:, i].rearrange("p c w -> p (c w)"),
                            Xb[(b, ch)][:, i].rearrange("p c w -> p (c w)"),
                            u2s[i])
                    sl = slice(ch * CH, (ch + 1) * CH)
                    # split across the two HWDGE rings: two concurrent streams
                    nc.scalar.dma_start(out=o2c[b][:, sl], in_=Cc)
                    nc.sync.dma_start(out=o1c[b][:, sl], in_=O1)

    nc.compile()
    return nc


def _fix_ties(x):
    """The device one-hot marks every channel equal to the max; the reference
    one_hot(argmax) marks only the first. Nudge later tied channels down by
    one ulp so a plain equality compare reproduces first-match semantics
    (out1 changes by <=1 ulp at those voxels)."""
    mx = x.max(axis=1, keepdims=True)
    ties = x == mx
    multi = ties.sum(axis=1) > 1
    if not multi.any():
        return x
    x = x.copy()
    for b, d, h, w in np.argwhere(multi):
        cs = np.flatnonzero(ties[b, :, d, h, w])
        for c in cs[1:]:
            x[b, c, d, h, w] = np.nextafter(x[b, c, d, h, w], -np.inf)
    return x


def _host_inputs(x, attentions):
    """Build per-core input maps from full inputs."""
    x = _fix_ties(x)
    att = attentions[..., 0].astype(np.float32) * np.float32(1.0 / 512.0)
    att_p = att.reshape(B, C, G, G, G).astype(ml_dtypes.bfloat16)
    pm = np.zeros((H, PATCH), dtype=ml_dtypes.bfloat16)
    pm[np.arange(H), np.arange(H) // POOL] = 1.0

    in_maps = []
    for core in range(NCORES):
        xs = x[:, :, core * DL:(core + 1) * DL]
        xt = np.ascontiguousarray(xs.transpose(0, 3, 2, 1, 4))  # [b,h,dl,c,w]
        # att2x[(ph,kh), b, c, pw, kw] = att_p[b, c, core, ph, pw]
        a = att_p[:, :, core]                         # [B, C, ph, pw]
        a2 = np.ascontiguousarray(np.broadcast_to(
            a.transpose(2, 0, 1, 3)[:, None, :, :, :, None],
            (G, PATCH, B, C, G, PATCH)).reshape(128, B, C, G, PATCH))
        in_maps.append({"xt": xt, "att2x": a2, "pmat": pm})
    return in_maps


def kernel(x, attentions):
    x = np.asarray(x, dtype=np.float32)
    attentions = np.asarray(attentions, dtype=np.float32)

    if "nc" not in _CACHE:
        _CACHE["nc"] = _build_nc()
    nc = _CACHE["nc"]

    in_maps = _host_inputs(x, attentions)
    res = bass_utils.run_bass_kernel_spmd(nc, in_maps,
                                          core_ids=list(range(NCORES)))

    out1 = np.empty((B, C, D, H, W), np.float32)
    out2 = np.empty((B, C, D, H, W), np.float32)
    for core in range(NCORES):
        sl = slice(core * DL, (core + 1) * DL)
        # [b,h,dl,c,w] -> [b,c,dl,h,w]
        out1[:, :, sl] = res.results[core]["o1c"].transpose(
            0, 3, 2, 1, 4).astype(np.float32)
        out2[:, :, sl] = res.results[core]["o2c"].transpose(
            0, 3, 2, 1, 4).astype(np.float32)
    return out1, out2
